# revision 22
# baseline (speedup 1.0000x reference)
"""Trainium2 Bass kernel for DeepTemplateMatchingModule.

Sharding: core c owns eval+template samples {2c, 2c+1} (data parallel); the
GRU cross-sample hidden chain is broken every L=16 positions and re-warmed
with W=32 steps.  conv1..conv3 compose into a single 1->64 13x13 conv.

v2: conv path in bf16 (4x PE rate, no fp32 throttle), conv rhs slabs are
im2col'd on the host and loaded as a few large contiguous DMAs (instead of
~350 small strided loads re-reading HBM 47x), P2g stores merged per k-iter,
margin units store only the 5 needed channels.
"""

import numpy as np
import ml_dtypes

bf16 = ml_dtypes.bfloat16

B, T, S, H = 16, 512, 496, 64
L = 16          # GRU chain length
W = 32          # warmup steps
GL = 16 * 64    # gi store cols per branch: t(16) x (2 margin + 62 chains)
KC = 29         # conv k iterations (4 W-outputs each)
NH = 500        # conv output h columns
KHALF = 15      # k iterations in first slab half

_CACHE = {}


def _compose_convs(w1, b1, w2, b2, w3, b3):
    def compose(wa, ba, wb, bb):
        O2, M, k2, _ = wb.shape
        _, I, k1, _ = wa.shape
        kc = k1 + k2 - 1
        wcm = np.zeros((O2, I, kc, kc), np.float64)
        wa64 = wa.astype(np.float64)
        wb64 = wb.astype(np.float64)
        for p in range(k2):
            for q in range(k2):
                wcm[:, :, p:p + k1, q:q + k1] += np.einsum(
                    'om,mikl->oikl', wb64[:, :, p, q], wa64)
        bcm = wb64.sum((2, 3)) @ ba.astype(np.float64) + bb
        return wcm, bcm

    wc12, bc12 = compose(w1, b1, w2, b2)
    wc, bc = compose(wc12, bc12, w3, b3)
    return wc[:, 0].astype(np.float32), bc.astype(np.float32)  # (64,13,13),(64,)


def _host_prep(inputs):
    wc, bc = _compose_convs(inputs['conv1_w'], inputs['conv1_b'],
                            inputs['conv2_w'], inputs['conv2_b'],
                            inputs['conv3_w'], inputs['conv3_b'])

    kh_i, dw_i = np.meshgrid(np.arange(13), np.arange(16), indexing='ij')

    def conv_lhsT(side):
        t1 = np.zeros((128, 128), np.float32)
        t2 = np.zeros((81, 128), np.float32)
        for jj in range(2):
            j = 2 * jj + side
            kw = dw_i - j
            ok = (kw >= 0) & (kw <= 12)
            for co in range(64):
                col = jj * 64 + co
                v = np.where(ok, wc[co][kh_i, np.clip(kw, 0, 12)], 0.0)
                t1[:, col] = v[:8].reshape(-1)
                t2[1:81, col] = v[8:].reshape(-1)
                t2[0, col] = bc[co]
        return t1.astype(bf16), t2.astype(bf16)

    convA1, convA2 = conv_lhsT(0)
    convB1, convB2 = conv_lhsT(1)

    L1 = inputs['lin1_w']
    lin1w = np.zeros((58, 64 * 64), np.float32)
    for cp in range(64):
        lin1w[:, cp * 64:(cp + 1) * 64] = L1[:, cp * 58:(cp + 1) * 58].T
    lin1w = lin1w.astype(bf16)
    # contraction-116 packing for non-margin units: block q = channels (2q, 2q+1)
    lin1w2 = np.zeros((116, 32 * 64), np.float32)
    for q in range(32):
        lin1w2[0:58, q * 64:(q + 1) * 64] = L1[:, (2 * q) * 58:(2 * q + 1) * 58].T
        lin1w2[58:116, q * 64:(q + 1) * 64] = L1[:, (2 * q + 1) * 58:(2 * q + 2) * 58].T
    lin1w2 = lin1w2.astype(bf16)
    lin1b = inputs['lin1_b'].reshape(1, 64).astype(np.float32)

    W_ih, b_ih = inputs['W_ih'], inputs['b_ih']
    W_hh, b_hh = inputs['W_hh'], inputs['b_hh']
    gruW_rz = np.concatenate([
        np.concatenate([W_hh[:64].T, b_hh[:64].reshape(1, 64)], 0),
        np.concatenate([W_hh[64:128].T, b_hh[64:128].reshape(1, 64)], 0)], 1)
    consts = dict(
        convA1=convA1, convA2=convA2, convB1=convB1, convB2=convB2,
        lin1w=lin1w, lin1w2=lin1w2, lin1b=lin1b,
        giW_r=np.ascontiguousarray(W_ih[:64].T).astype(np.float32),
        giW_z=np.ascontiguousarray(W_ih[64:128].T).astype(np.float32),
        giW_n=np.ascontiguousarray(W_ih[128:].T).astype(np.float32),
        giB_r=b_ih[:64].reshape(1, 64).astype(np.float32),
        giB_z=b_ih[64:128].reshape(1, 64).astype(np.float32),
        giB_n=b_ih[128:].reshape(1, 64).astype(np.float32),
        gruW_rz=gruW_rz.astype(np.float32),
        gruW_n=np.concatenate([W_hh[128:].T, b_hh[128:].reshape(1, 64)], 0).astype(np.float32),
        attw=np.ascontiguousarray(inputs['att_w'].reshape(1, 64).T).astype(np.float32),
        lin3T=np.ascontiguousarray(inputs['lin3_w'].T).astype(np.float32),
        lin3b=inputs['lin3_b'].reshape(1, 128).astype(np.float32),
        clsT=np.ascontiguousarray(inputs['cls_w'].T).astype(np.float32),
        clsb=inputs['cls_b'].reshape(1, 2).astype(np.float32),
    )

    def im2col(x):
        # x: (128, 512) f32.  Returns s1 (29, 128, 500), s2 (29, 80, 500):
        # s1[k, kh*16+dw, h] = x[4k+dw, kh+h]        kh in 0..7
        # s2[k, kh2*16+dw, h] = x[4k+dw, 8+kh2+h]    kh2 in 0..4
        st = x.strides  # (2048, 4) bytes
        s1 = np.lib.stride_tricks.as_strided(
            x, shape=(KC, 8, 16, NH), strides=(4 * st[0], st[1], st[0], st[1]))
        s2 = np.lib.stride_tricks.as_strided(
            x[:, 8:], shape=(KC, 5, 16, NH), strides=(4 * st[0], st[1], st[0], st[1]))
        return (s1.reshape(KC, 128, NH), s2.reshape(KC, 80, NH))

    ev, tm = inputs['evaluation'], inputs['template']
    in_maps = []
    for c in range(8):
        x6 = np.zeros((6, 128, 512), np.float32)
        if c > 0:
            x6[0] = ev[2 * c - 1]
            x6[3] = tm[2 * c - 1]
        x6[1], x6[2] = ev[2 * c], ev[2 * c + 1]
        x6[4], x6[5] = tm[2 * c], tm[2 * c + 1]

        slab1 = np.zeros((12, 128, KHALF * NH), bf16)
        slab2 = np.zeros((12, 81, KHALF * NH), bf16)
        slab2[:, 0, :] = bf16(1.0)
        for u in range(6):
            s1, s2 = im2col(x6[u])
            for h in range(2):
                k0, k1 = (0, KHALF) if h == 0 else (KHALF, KC)
                n = (k1 - k0) * NH
                slab1[2 * u + h, :, :n] = (
                    s1[k0:k1].transpose(1, 0, 2).reshape(128, n).astype(bf16))
                slab2[2 * u + h, 1:81, :n] = (
                    s2[k0:k1].transpose(1, 0, 2).reshape(80, n).astype(bf16))

        kill = np.ones((64, 124), np.float32)
        if c == 0:
            kill[:, 0] = 0.0
            kill[:, 62] = 0.0
        m = dict(consts)
        m['slab1'] = slab1
        m['slab2'] = slab2
        m['kill'] = kill
        in_maps.append(m)
    return in_maps


def _ap_mod(ap, dims, extra_offset=0):
    """Rebuild an AP keeping its partition dim, custom free dims, offset shift."""
    import dataclasses
    d0 = [ap.ap[0][0], ap.ap[0][1]]
    return dataclasses.replace(ap, ap=[d0] + [list(d) for d in dims],
                               offset=ap.offset + extra_offset)


# ---------------------------------------------------------------------------
# Walrus workaround: this toolchain's codegen accepts at most ONE sem-wait per
# instruction ("Too many sync wait commands"), but Tile emits several.  Split
# every instruction with N>1 waits into N-1 preceding same-engine NoOps
# carrying one wait each, applied to the BIR json just before compile.
def _split_waits_bir(bir_bytes):
    import orjson
    m = orjson.loads(bir_bytes)
    ctr = [0]
    for fn in m['functions']:
        for bb in fn.get('blocks') or []:
            insts = bb.get('instructions')
            if not insts:
                continue
            out = []
            for inst in insts:
                si = inst.get('sync_info')
                waits = (si or {}).get('on_wait') or []
                if len(waits) > 1:
                    for w in waits[:-1]:
                        ctr[0] += 1
                        out.append({
                            'name': "%s_sw%d" % (inst['name'], ctr[0]),
                            'opcode': 'NoOp',
                            'engine': inst['engine'],
                            'ins': [], 'outs': [],
                            'debug': inst.get('debug'),
                            'sync_info': {'on_update': [], 'on_wait': [w]},
                        })
                    si['on_wait'] = [waits[-1]]
                out.append(inst)
            bb['instructions'] = out
    return orjson.dumps(m)


def _install_bir_fix():
    if _CACHE.get('bir_fix'):
        return
    _CACHE['bir_fix'] = True
    import concourse.bass2jax as b2j
    import concourse.bass_utils as bu
    orig = bu.compile_bir_kernel

    def wrapped(bir_json, tmpdir, neff_name='file.neff'):
        if isinstance(bir_json, str):
            bir_json = bir_json.encode()
        return orig(_split_waits_bir(bir_json), tmpdir, neff_name=neff_name)

    b2j.compile_bir_kernel = wrapped
    bu.compile_bir_kernel = wrapped


def _build_program():
    import concourse.bass as bass
    import concourse.mybir as mybir
    import concourse.tile as tile
    from concourse.masks import make_identity

    f32 = mybir.dt.float32
    bft = mybir.dt.bfloat16
    AF = mybir.ActivationFunctionType
    ALU = mybir.AluOpType

    nc = bass.Bass()

    din = {}
    for name, shape, dt in [
        ('slab1', (12, 128, KHALF * NH), bft),
        ('slab2', (12, 81, KHALF * NH), bft),
        ('kill', (64, 124), f32),
        ('convA1', (128, 128), bft), ('convA2', (81, 128), bft),
        ('convB1', (128, 128), bft), ('convB2', (81, 128), bft),
        ('lin1w', (58, 4096), bft), ('lin1w2', (116, 2048), bft),
        ('lin1b', (1, 64), f32),
        ('giW_r', (64, 64), f32), ('giW_z', (64, 64), f32), ('giW_n', (64, 64), f32),
        ('giB_r', (1, 64), f32), ('giB_z', (1, 64), f32), ('giB_n', (1, 64), f32),
        ('gruW_rz', (65, 128), f32), ('gruW_n', (65, 64), f32),
        ('attw', (64, 1), f32), ('lin3T', (64, 128), f32),
        ('lin3b', (1, 128), f32), ('clsT', (128, 2), f32),
        ('clsb', (1, 2), f32),
    ]:
        din[name] = nc.declare_dram_parameter(name, list(shape), dt, isOutput=False)
    dout = nc.declare_dram_parameter('outloc', [2, 2], f32, isOutput=True)

    with tile.TileContext(nc) as tc:
        with tc.tile_pool(name='persist', bufs=1) as pp:
            # rows 0..57: col 1+l holds pooled-linear element l (l = 496c+s)
            # rows 58..115: col l holds element l (the +1-shifted copy), so a
            # single read column x gives element x-1 on top and x on bottom.
            P2g = pp.tile([116, 31746], bft)
            Pm = pp.tile([58, 2480], bft)
            gi_rz = pp.tile([128, 2 * GL], f32)
            gi_n = pp.tile([64, 2 * GL], f32)
            ETm = pp.tile([65, 2048], f32)
            hA = pp.tile([65, 124], f32)
            hB = pp.tile([65, 124], f32)
            ident = pp.tile([128, 128], f32)
            ones = pp.tile([1, 512], f32)
            lin1out = pp.tile([64, 496], f32)

            cst = {}
            for name in ['convA1', 'convA2', 'convB1', 'convB2',
                         'lin1w', 'lin1w2', 'lin1b', 'giW_r', 'giW_z', 'giW_n',
                         'giB_r', 'giB_z', 'giB_n', 'gruW_rz',
                         'gruW_n', 'attw', 'lin3T', 'lin3b', 'clsT', 'clsb',
                         'kill']:
                t = pp.tile(list(din[name].shape), din[name].dtype, name=f'c_{name}')
                nc.sync.dma_start(t, din[name][:, :])
                cst[name] = t

            make_identity(nc, ident)
            nc.vector.memset(ones, 1.0)
            nc.vector.memset(ETm[64:65, :], 1.0)
            nc.vector.memset(hA, 0.0)
            nc.vector.memset(hA[64:65, :], 1.0)
            nc.vector.memset(hB[64:65, :], 1.0)

            # ================= PHASE A =================
            with tc.tile_pool(name='pA', bufs=3) as pa, \
                 tc.tile_pool(name='pAs', bufs=2) as psl, \
                 tc.tile_pool(name='pAp', bufs=3, space='PSUM') as pap, \
                 tc.tile_pool(name='pAl', bufs=1, space='PSUM') as pal:

                def conv_unit(u, margin):
                    dst = Pm if margin else P2g
                    for half in range(2):
                        nk = KHALF if half == 0 else KC - KHALF
                        s1 = psl.tile([128, KHALF * NH], bft, tag='ms1')
                        s2 = psl.tile([81, KHALF * NH], bft, tag='ms2',
                                      padded_shape=[128, KHALF * NH])
                        nc.sync.dma_start(s1, din['slab1'][2 * u + half])
                        nc.sync.dma_start(s2, din['slab2'][2 * u + half])
                        for kp in range(0, nk, 2):   # pairs of k iterations
                            npair = min(2, nk - kp)
                            pwc = pa.tile([128, 2 * NH], bft, tag='pwc')
                            for par in range(npair):
                                kl = kp + par
                                r1 = s1[:, kl * NH:(kl + 1) * NH]
                                r2 = s2[0:81, kl * NH:(kl + 1) * NH]
                                psA = pap.tile([128, NH], f32, tag='cps')
                                psB = pap.tile([128, NH], f32, tag='cps')
                                nc.tensor.matmul(psA, cst['convA1'], r1, start=True, stop=False)
                                nc.tensor.matmul(psA, cst['convA2'], r2, start=False, stop=True)
                                nc.tensor.matmul(psB, cst['convB1'], r1, start=True, stop=False)
                                nc.tensor.matmul(psB, cst['convB2'], r2, start=False, stop=True)
                                pwb = pa.tile([128, NH], bft, tag='pwb')
                                nc.scalar.activation(pwb, psB, AF.Copy)
                                nc.vector.tensor_tensor(
                                    pwc[:, par * NH:(par + 1) * NH], psA, pwb, op=ALU.max)
                            w = npair * NH
                            m2 = pa.tile([128, 2 * NH], bft, tag='m2', name='m2')[:, 0:w - 1]
                            nc.vector.tensor_tensor(m2, pwc[:, 0:w - 1], pwc[:, 1:w], op=ALU.max)
                            m4 = pa.tile([128, 2 * NH], bft, tag='m4', name='m4')[:, 0:w - 3]
                            nc.vector.tensor_tensor(m4, m2[:, 0:w - 3], m2[:, 2:w - 1], op=ALU.max)
                            pooled = pa.tile([128, 2 * NH], bft, tag='pool', name='pooled')[:, 0:w - 4]
                            nc.vector.tensor_tensor(pooled, m4[:, 0:w - 4], pwc[:, 4:w], op=ALU.max)
                            for par in range(npair):
                                k = (kp + par) if half == 0 else KHALF + kp + par
                                src = pooled[:, par * NH:par * NH + 496]
                                if margin:
                                    # only channels 59..63: Pm[w, (c-59)*496+s]
                                    for jj in range(2):
                                        wrow = 2 * k + jj
                                        nc.gpsimd.dma_start(
                                            _ap_mod(Pm[wrow:wrow + 1, 0:1],
                                                    [[496, 5], [1, 496]]),
                                            src[jj * 64 + 59:jj * 64 + 64, :])
                                else:
                                    # top half: P2g[2k+jj, 1 + 496c + s]
                                    nc.gpsimd.dma_start(
                                        _ap_mod(P2g[2 * k:2 * k + 2, 0:1],
                                                [[496, 64], [1, 496]], 1),
                                        src[0:128, :])
                                    # shifted half: P2g[58+2k+jj, 496c + s]
                                    nc.gpsimd.dma_start(
                                        _ap_mod(P2g[58 + 2 * k:58 + 2 * k + 2, 0:1],
                                                [[496, 64], [1, 496]]),
                                        src[0:128, :])

                def lin_gi(br, b_loc, margin):
                    n = 32 if margin else 496
                    pl = pal.tile([64, 512], f32, tag='linps', name='pl')[:, 0:n]
                    if margin:
                        for cp in range(64):
                            rhs = _ap_mod(Pm[0:58, 0:1], [[64, n]], 432 + cp)
                            nc.tensor.matmul(pl, cst['lin1w'][:, cp * 64:(cp + 1) * 64],
                                             rhs, start=(cp == 0), stop=False)
                    else:
                        for q in range(32):
                            rhs = _ap_mod(P2g[0:116, 0:1], [[64, n]], 1 + 2 * q)
                            nc.tensor.matmul(pl, cst['lin1w2'][:, q * 64:(q + 1) * 64],
                                             rhs, start=(q == 0), stop=False)
                    nc.tensor.matmul(pl, cst['lin1b'], ones[0:1, 0:n],
                                     start=False, stop=True)
                    lo = lin1out[:, 0:n]
                    nc.scalar.activation(lo, pl, AF.Copy)
                    gparts = [('giW_r', 'giB_r', gi_rz, 0, 'gir'),
                              ('giW_z', 'giB_z', gi_rz, 64, 'giz'),
                              ('giW_n', 'giB_n', gi_n, 0, 'gin')]
                    for wname, bname, store, prow, tg in gparts:
                        pg = pal.tile([64, 512], f32, tag=tg, name='pg_' + tg)[:, 0:n]
                        nc.tensor.matmul(pg, cst[wname], lo, start=True, stop=False)
                        nc.tensor.matmul(pg, cst[bname], ones[0:1, 0:n], start=False, stop=True)
                        if margin:
                            # psum col i = (grp, t): dst col = t*64 + grp
                            nc.scalar.activation(
                                _ap_mod(store[prow:prow + 64, 0:1], [[1, 2], [64, 16]], br * GL),
                                pg, AF.Copy)
                        else:
                            # psum col s' = (j, t): dst col = t*64 + (31*b_loc + j + 2)
                            off = br * GL + 31 * b_loc + 2
                            nc.scalar.activation(
                                _ap_mod(store[prow:prow + 64, 0:1], [[1, 31], [64, 16]], off),
                                _ap_mod(pg, [[16, 31], [1, 16]]), AF.Copy)

                for u, (br, b_loc, margin) in enumerate([
                        (0, 0, True), (0, 0, False), (0, 1, False),
                        (1, 0, True), (1, 0, False), (1, 1, False)]):
                    conv_unit(u, margin)
                    lin_gi(br, b_loc, margin)

            # ================= PHASE B: GRU =================
            # the two branches (eval / template) are independent chains; they
            # are emitted as separate 62-col streams so their steps interleave
            # across engines and hide the per-step handoff latency.
            with tc.tile_pool(name='pB', bufs=2) as pb, \
                 tc.tile_pool(name='pBp', bufs=2, space='PSUM') as pbp:

                def h_ap(br, t, p=65):
                    return _ap_mod(ETm[0:p, 0:1], [[16, 62]], br * 1024 + t)

                def gi_ap(store, br, p, t):
                    if t >= 0:
                        tp, dlt = t, 0
                    elif t >= -16:
                        tp, dlt = 16 + t, -1
                    else:
                        tp, dlt = 32 + t, -2
                    return _ap_mod(store[0:p, 0:1], [[1, 62]],
                                   br * GL + tp * 64 + 2 + dlt)

                killed = pp.tile([65, 124], f32)
                nc.vector.memset(killed[64:65, :], 1.0)

                for i, t in enumerate(range(-W, L)):
                    for br in range(2):
                        cs = slice(br * 62, br * 62 + 62)
                        if t < 0:
                            h_in = hA if i % 2 == 0 else hB
                            h_out_ap = (hB if i % 2 == 0 else hA)[0:64, cs]
                        elif t == 0:
                            h_in = killed
                            h_out_ap = h_ap(br, 0, 64)
                        else:
                            h_in = None
                            h_out_ap = h_ap(br, t, 64)

                        h_in_ap = h_in[0:65, cs] if h_in is not None else h_ap(br, t - 1)
                        h_in64 = h_in[0:64, cs] if h_in is not None else h_ap(br, t - 1, 64)

                        prz = pbp.tile([128, 62], f32, tag=f'grz{br}', name='prz',
                                       padded_shape=[128, 64])
                        pn = pbp.tile([64, 62], f32, tag=f'gn{br}', name='pn',
                                      padded_shape=[128, 64])
                        nc.tensor.matmul(prz, cst['gruW_rz'], h_in_ap, start=True, stop=False)
                        nc.tensor.matmul(prz, ident, gi_ap(gi_rz, br, 128, t),
                                         start=False, stop=True)
                        nc.tensor.matmul(pn, cst['gruW_n'], h_in_ap, start=True, stop=True)
                        r = pb.tile([64, 62], f32, tag=f'r{br}', name='r',
                                    padded_shape=[128, 62])
                        nc.scalar.activation(r, prz[0:64, :], AF.Sigmoid)
                        z = pb.tile([64, 62], f32, tag=f'z{br}', name='z',
                                    padded_shape=[128, 62])
                        nc.scalar.activation(z, prz[64:128, :], AF.Sigmoid)
                        t2 = pb.tile([64, 62], f32, tag=f't2{br}', name='t2',
                                     padded_shape=[128, 62])
                        nc.vector.tensor_mul(t2, r, pn)
                        npre = pb.tile([64, 62], f32, tag=f'npre{br}', name='npre',
                                       padded_shape=[128, 62])
                        nc.vector.tensor_add(npre, t2, gi_ap(gi_n, br, 64, t))
                        nt = pb.tile([64, 62], f32, tag=f'nt{br}', name='nt',
                                     padded_shape=[128, 62])
                        nc.scalar.activation(nt, npre, AF.Tanh)
                        dmn = pb.tile([64, 62], f32, tag=f'dmn{br}', name='dmn',
                                      padded_shape=[128, 62])
                        nc.vector.scalar_tensor_tensor(dmn, nt, -1.0, h_in64,
                                                       op0=ALU.mult, op1=ALU.add)
                        e = pb.tile([64, 62], f32, tag=f'e{br}', name='e',
                                    padded_shape=[128, 62])
                        nc.vector.tensor_mul(e, z, dmn)
                        nc.vector.tensor_add(h_out_ap, nt, e)
                        if t == -1:
                            last = hB if i % 2 == 0 else hA
                            nc.vector.tensor_mul(killed[0:64, cs], last[0:64, cs],
                                                 cst['kill'][:, cs])

            # ================= PHASE C =================
            with tc.tile_pool(name='pC', bufs=2) as pc_, \
                 tc.tile_pool(name='pCe', bufs=4) as pce, \
                 tc.tile_pool(name='pCp', bufs=2, space='PSUM') as pcp, \
                 tc.tile_pool(name='pCs', bufs=1, space='PSUM') as pcs:
                NB = [(0, 128), (128, 128), (256, 128), (384, 112)]
                for b_loc in range(2):
                    Es = ETm[0:64, b_loc * 496:b_loc * 496 + 496]
                    Ts = ETm[0:64, 1024 + b_loc * 496:1024 + b_loc * 496 + 496]
                    etiles, tmts = [], []
                    for (nb0, nbs) in NB:
                        psc = pcp.tile([128, 512], f32, tag='sc', name='psc')[0:nbs, 0:496]
                        nc.tensor.matmul(psc, Ts[:, nb0:nb0 + nbs], Es, start=True, stop=True)
                        nmx = pc_.tile([128, 1], f32, tag='nmx', name='nmx')[0:nbs, :]
                        nc.vector.tensor_reduce(nmx, psc, axis=mybir.AxisListType.X,
                                                op=ALU.max, negate=True)
                        et = pce.tile([128, 496], bft, tag='et', name='et')[0:nbs, :]
                        ssum = pc_.tile([128, 1], f32, tag='ssum', name='ssum')[0:nbs, :]
                        nc.scalar.activation(et, psc, AF.Exp, bias=nmx, accum_out=ssum)
                        rs = pc_.tile([128, 1], f32, tag='rs', name='rs')[0:nbs, :]
                        nc.vector.reciprocal(rs, ssum)
                        ptt = pcp.tile([128, 512], f32, tag='ptt', bufs=1, name='ptt')[0:nbs, 0:64]
                        nc.tensor.transpose(ptt, Ts[:, nb0:nb0 + nbs], ident[0:64, 0:64])
                        tmt = pce.tile([128, 64], bft, tag='tmt', name='tmt')[0:nbs, :]
                        nc.scalar.activation(tmt, ptt, AF.Copy, scale=rs)
                        etiles.append(et)
                        tmts.append(tmt)
                    ptp = pcs.tile([64, 512], f32, tag='tp', padded_shape=[128, 512], name='ptp')[:, 0:496]
                    for q, (nb0, nbs) in enumerate(NB):
                        nc.tensor.matmul(ptp, tmts[q], etiles[q],
                                         start=(q == 0), stop=(q == 3))
                    da = pc_.tile([64, 496], f32, tag='da', padded_shape=[128, 496])
                    nc.vector.tensor_sub(da, ptp, Es)
                    da2 = pc_.tile([64, 496], f32, tag='da2', padded_shape=[128, 496])
                    nc.scalar.activation(da2, da, AF.Abs)
                    patt = pcs.tile([1, 512], f32, tag='chain', padded_shape=[128, 512], bufs=2, name='patt')[:, 0:496]
                    nc.tensor.matmul(patt, cst['attw'], Es, start=True, stop=True)
                    anm = pc_.tile([1, 1], f32, tag='anm', padded_shape=[128, 1])
                    nc.vector.tensor_reduce(anm, patt, axis=mybir.AxisListType.X,
                                            op=ALU.max, negate=True)
                    ea = pc_.tile([1, 496], f32, tag='ea', padded_shape=[128, 496])
                    asum = pc_.tile([1, 1], f32, tag='asum', padded_shape=[128, 1])
                    nc.scalar.activation(ea, patt, AF.Exp, bias=anm, accum_out=asum)
                    ars = pc_.tile([1, 1], f32, tag='ars', padded_shape=[128, 1])
                    nc.vector.reciprocal(ars, asum)
                    pab = pcs.tile([64, 512], f32, tag='pab', padded_shape=[128, 512], name='pab')[:, 0:496]
                    nc.tensor.matmul(pab, ones[0:1, 0:64], ea, start=True, stop=True)
                    junk = pc_.tile([64, 496], f32, tag='junk', padded_shape=[128, 496])
                    nc.vector.tensor_mul(junk, da2, pab)
                    rep = pc_.tile([64, 1], f32, tag='rep', padded_shape=[128, 1])
                    nc.vector.tensor_reduce(rep, junk, axis=mybir.AxisListType.X,
                                            op=ALU.add)
                    prsb = pcs.tile([64, 1], f32, tag='chain', padded_shape=[128, 512], bufs=2)
                    nc.tensor.matmul(prsb, ones[0:1, 0:64], ars, start=True, stop=True)
                    rsb = pc_.tile([64, 1], f32, tag='rsb', padded_shape=[128, 1])
                    nc.scalar.activation(rsb, prsb, AF.Copy)
                    h1 = pc_.tile([64, 1], f32, tag='h1', padded_shape=[128, 1])
                    nc.scalar.activation(h1, rep, AF.Relu, scale=rsb)
                    ph2 = pcs.tile([128, 1], f32, tag='chain', padded_shape=[128, 512], bufs=2)
                    nc.tensor.matmul(ph2, cst['lin3T'], h1, start=True, stop=False)
                    nc.tensor.matmul(ph2, cst['lin3b'], ones[0:1, 0:1], start=False, stop=True)
                    h2 = pc_.tile([128, 1], f32, tag='h2')
                    nc.scalar.activation(h2, ph2, AF.Relu)
                    po = pcs.tile([2, 1], f32, tag='chain', padded_shape=[128, 512], bufs=2)
                    nc.tensor.matmul(po, cst['clsT'], h2, start=True, stop=False)
                    nc.tensor.matmul(po, cst['clsb'], ones[0:1, 0:1], start=False, stop=True)
                    osb = pc_.tile([2, 1], f32, tag='osb', padded_shape=[128, 1])
                    nc.scalar.activation(osb, po, AF.Copy)
                    nc.sync.dma_start(dout[b_loc:b_loc + 1, 0:2], osb)

    return nc


def kernel(**inputs):
    _install_bir_fix()
    inputs = {k: np.asarray(v) for k, v in inputs.items()}
    in_maps = _host_prep(inputs)
    if 'nc' not in _CACHE:
        _CACHE['nc'] = _build_program()
    nc = _CACHE['nc']
    from concourse.bass_utils import run_bass_kernel_spmd
    res = run_bass_kernel_spmd(nc, in_maps, core_ids=list(range(8)))
    out = np.zeros((16, 2), np.float32)
    for c in range(8):
        out[2 * c:2 * c + 2] = res.results[c]['outloc']
    return out


# revision 26
# speedup vs baseline: 1.0899x; 1.0899x over previous
"""Trainium2 Bass kernel for DeepTemplateMatchingModule.

Sharding: core c owns eval+template samples {2c, 2c+1} (data parallel); the
GRU cross-sample hidden chain is broken every L=16 positions and re-warmed
with W=32 steps.  conv1..conv3 compose into a single 1->64 13x13 conv.

v2: conv path in bf16 (4x PE rate, no fp32 throttle), conv rhs slabs are
im2col'd on the host and loaded as a few large contiguous DMAs (instead of
~350 small strided loads re-reading HBM 47x), P2g stores merged per k-iter,
margin units store only the 5 needed channels.
"""

import numpy as np
import ml_dtypes

bf16 = ml_dtypes.bfloat16

B, T, S, H = 16, 512, 496, 64
L = 16          # GRU chain length
W = 32          # warmup steps
GL = 16 * 64    # gi store cols per branch: t(16) x (2 margin + 62 chains)
KC = 29         # conv k iterations (4 W-outputs each)
NH = 500        # conv output h columns
KHALF = 15      # k iterations in first slab half

_CACHE = {}


def _compose_convs(w1, b1, w2, b2, w3, b3):
    def compose(wa, ba, wb, bb):
        O2, M, k2, _ = wb.shape
        _, I, k1, _ = wa.shape
        kc = k1 + k2 - 1
        wcm = np.zeros((O2, I, kc, kc), np.float64)
        wa64 = wa.astype(np.float64)
        wb64 = wb.astype(np.float64)
        for p in range(k2):
            for q in range(k2):
                wcm[:, :, p:p + k1, q:q + k1] += np.einsum(
                    'om,mikl->oikl', wb64[:, :, p, q], wa64)
        bcm = wb64.sum((2, 3)) @ ba.astype(np.float64) + bb
        return wcm, bcm

    wc12, bc12 = compose(w1, b1, w2, b2)
    wc, bc = compose(wc12, bc12, w3, b3)
    return wc[:, 0].astype(np.float32), bc.astype(np.float32)  # (64,13,13),(64,)


def _host_prep(inputs):
    wc, bc = _compose_convs(inputs['conv1_w'], inputs['conv1_b'],
                            inputs['conv2_w'], inputs['conv2_b'],
                            inputs['conv3_w'], inputs['conv3_b'])

    kh_i, dw_i = np.meshgrid(np.arange(13), np.arange(16), indexing='ij')

    def conv_lhsT(side):
        t1 = np.zeros((128, 128), np.float32)
        t2 = np.zeros((81, 128), np.float32)
        for jj in range(2):
            j = 2 * jj + side
            kw = dw_i - j
            ok = (kw >= 0) & (kw <= 12)
            for co in range(64):
                col = jj * 64 + co
                v = np.where(ok, wc[co][kh_i, np.clip(kw, 0, 12)], 0.0)
                t1[:, col] = v[:8].reshape(-1)
                t2[1:81, col] = v[8:].reshape(-1)
                t2[0, col] = bc[co]
        return t1.astype(bf16), t2.astype(bf16)

    convA1, convA2 = conv_lhsT(0)
    convB1, convB2 = conv_lhsT(1)

    L1 = inputs['lin1_w']
    lin1w = np.zeros((58, 64 * 64), np.float32)
    for cp in range(64):
        lin1w[:, cp * 64:(cp + 1) * 64] = L1[:, cp * 58:(cp + 1) * 58].T
    lin1w = lin1w.astype(bf16)
    # contraction-122 packing for non-margin units: block q = channels (2q, 2q+1)
    # rows 58..63 are zero (the shifted copy lives at partitions 64..121 so the
    # engine writes stay 32-aligned)
    lin1w2 = np.zeros((122, 32 * 64), np.float32)
    for q in range(32):
        lin1w2[0:58, q * 64:(q + 1) * 64] = L1[:, (2 * q) * 58:(2 * q + 1) * 58].T
        lin1w2[64:122, q * 64:(q + 1) * 64] = L1[:, (2 * q + 1) * 58:(2 * q + 2) * 58].T
    lin1w2 = lin1w2.astype(bf16)
    lin1b = inputs['lin1_b'].reshape(1, 64).astype(np.float32)

    W_ih, b_ih = inputs['W_ih'], inputs['b_ih']
    W_hh, b_hh = inputs['W_hh'], inputs['b_hh']
    gruW_rz = np.concatenate([
        np.concatenate([W_hh[:64].T, b_hh[:64].reshape(1, 64)], 0),
        np.concatenate([W_hh[64:128].T, b_hh[64:128].reshape(1, 64)], 0)], 1)
    consts = dict(
        convA1=convA1, convA2=convA2, convB1=convB1, convB2=convB2,
        lin1w=lin1w, lin1w2=lin1w2, lin1b=lin1b,
        giW_r=np.ascontiguousarray(W_ih[:64].T).astype(np.float32),
        giW_z=np.ascontiguousarray(W_ih[64:128].T).astype(np.float32),
        giW_n=np.ascontiguousarray(W_ih[128:].T).astype(np.float32),
        giB_r=b_ih[:64].reshape(1, 64).astype(np.float32),
        giB_z=b_ih[64:128].reshape(1, 64).astype(np.float32),
        giB_n=b_ih[128:].reshape(1, 64).astype(np.float32),
        gruW_rz=gruW_rz.astype(np.float32),
        gruW_n=np.concatenate([W_hh[128:].T, b_hh[128:].reshape(1, 64)], 0).astype(np.float32),
        attw=np.ascontiguousarray(inputs['att_w'].reshape(1, 64).T).astype(np.float32),
        lin3T=np.ascontiguousarray(inputs['lin3_w'].T).astype(np.float32),
        lin3b=inputs['lin3_b'].reshape(1, 128).astype(np.float32),
        clsT=np.ascontiguousarray(inputs['cls_w'].T).astype(np.float32),
        clsb=inputs['cls_b'].reshape(1, 2).astype(np.float32),
    )

    def im2col(x):
        # x: (128, 512) f32.  Returns s1 (29, 128, 500), s2 (29, 80, 500):
        # s1[k, kh*16+dw, h] = x[4k+dw, kh+h]        kh in 0..7
        # s2[k, kh2*16+dw, h] = x[4k+dw, 8+kh2+h]    kh2 in 0..4
        st = x.strides  # (2048, 4) bytes
        s1 = np.lib.stride_tricks.as_strided(
            x, shape=(KC, 8, 16, NH), strides=(4 * st[0], st[1], st[0], st[1]))
        s2 = np.lib.stride_tricks.as_strided(
            x[:, 8:], shape=(KC, 5, 16, NH), strides=(4 * st[0], st[1], st[0], st[1]))
        return (s1.reshape(KC, 128, NH), s2.reshape(KC, 80, NH))

    ev, tm = inputs['evaluation'], inputs['template']
    in_maps = []
    for c in range(8):
        x6 = np.zeros((6, 128, 512), np.float32)
        if c > 0:
            x6[0] = ev[2 * c - 1]
            x6[3] = tm[2 * c - 1]
        x6[1], x6[2] = ev[2 * c], ev[2 * c + 1]
        x6[4], x6[5] = tm[2 * c], tm[2 * c + 1]

        slab1 = np.zeros((12, 128, KHALF * NH), bf16)
        slab2 = np.zeros((12, 81, KHALF * NH), bf16)
        slab2[:, 0, :] = bf16(1.0)
        for u in range(6):
            s1, s2 = im2col(x6[u])
            for h in range(2):
                k0, k1 = (0, KHALF) if h == 0 else (KHALF, KC)
                n = (k1 - k0) * NH
                slab1[2 * u + h, :, :n] = (
                    s1[k0:k1].transpose(1, 0, 2).reshape(128, n).astype(bf16))
                slab2[2 * u + h, 1:81, :n] = (
                    s2[k0:k1].transpose(1, 0, 2).reshape(80, n).astype(bf16))

        kill = np.ones((64, 124), np.float32)
        if c == 0:
            kill[:, 0] = 0.0
            kill[:, 62] = 0.0
        m = dict(consts)
        m['slab1'] = slab1
        m['slab2'] = slab2
        m['kill'] = kill
        in_maps.append(m)
    return in_maps


def _ap_mod(ap, dims, extra_offset=0):
    """Rebuild an AP keeping its partition dim, custom free dims, offset shift."""
    import dataclasses
    d0 = [ap.ap[0][0], ap.ap[0][1]]
    return dataclasses.replace(ap, ap=[d0] + [list(d) for d in dims],
                               offset=ap.offset + extra_offset)


# ---------------------------------------------------------------------------
# Walrus workaround: this toolchain's codegen accepts at most ONE sem-wait per
# instruction ("Too many sync wait commands"), but Tile emits several.  Split
# every instruction with N>1 waits into N-1 preceding same-engine NoOps
# carrying one wait each, applied to the BIR json just before compile.
def _split_waits_bir(bir_bytes):
    import orjson
    m = orjson.loads(bir_bytes)
    ctr = [0]
    for fn in m['functions']:
        for bb in fn.get('blocks') or []:
            insts = bb.get('instructions')
            if not insts:
                continue
            out = []
            for inst in insts:
                si = inst.get('sync_info')
                waits = (si or {}).get('on_wait') or []
                if len(waits) > 1:
                    for w in waits[:-1]:
                        ctr[0] += 1
                        out.append({
                            'name': "%s_sw%d" % (inst['name'], ctr[0]),
                            'opcode': 'NoOp',
                            'engine': inst['engine'],
                            'ins': [], 'outs': [],
                            'debug': inst.get('debug'),
                            'sync_info': {'on_update': [], 'on_wait': [w]},
                        })
                    si['on_wait'] = [waits[-1]]
                out.append(inst)
            bb['instructions'] = out
    return orjson.dumps(m)


def _install_bir_fix():
    if _CACHE.get('bir_fix'):
        return
    _CACHE['bir_fix'] = True
    import concourse.bass2jax as b2j
    import concourse.bass_utils as bu
    orig = bu.compile_bir_kernel

    def wrapped(bir_json, tmpdir, neff_name='file.neff'):
        if isinstance(bir_json, str):
            bir_json = bir_json.encode()
        return orig(_split_waits_bir(bir_json), tmpdir, neff_name=neff_name)

    b2j.compile_bir_kernel = wrapped
    bu.compile_bir_kernel = wrapped


def _build_program():
    import concourse.bass as bass
    import concourse.mybir as mybir
    import concourse.tile as tile
    from concourse.masks import make_identity

    f32 = mybir.dt.float32
    bft = mybir.dt.bfloat16
    AF = mybir.ActivationFunctionType
    ALU = mybir.AluOpType

    nc = bass.Bass()

    din = {}
    for name, shape, dt in [
        ('slab1', (12, 128, KHALF * NH), bft),
        ('slab2', (12, 81, KHALF * NH), bft),
        ('kill', (64, 124), f32),
        ('convA1', (128, 128), bft), ('convA2', (81, 128), bft),
        ('convB1', (128, 128), bft), ('convB2', (81, 128), bft),
        ('lin1w', (58, 4096), bft), ('lin1w2', (122, 2048), bft),
        ('lin1b', (1, 64), f32),
        ('giW_r', (64, 64), f32), ('giW_z', (64, 64), f32), ('giW_n', (64, 64), f32),
        ('giB_r', (1, 64), f32), ('giB_z', (1, 64), f32), ('giB_n', (1, 64), f32),
        ('gruW_rz', (65, 128), f32), ('gruW_n', (65, 64), f32),
        ('attw', (64, 1), f32), ('lin3T', (64, 128), f32),
        ('lin3b', (1, 128), f32), ('clsT', (128, 2), f32),
        ('clsb', (1, 2), f32),
    ]:
        din[name] = nc.declare_dram_parameter(name, list(shape), dt, isOutput=False)
    dout = nc.declare_dram_parameter('outloc', [2, 2], f32, isOutput=True)

    with tile.TileContext(nc) as tc:
        with tc.tile_pool(name='persist', bufs=1) as pp:
            # rows 0..57: col 1+l holds pooled-linear element l (l = 496c+s)
            # rows 64..121: col l holds element l (the +1-shifted copy), so a
            # single read column x gives element x-1 on top and x on bottom.
            P2g = pp.tile([122, 31746], bft)
            Pm = pp.tile([58, 2480], bft)
            gi_rz = pp.tile([128, 2 * GL], f32)
            gi_n = pp.tile([64, 2 * GL], f32)
            ETm = pp.tile([65, 2048], f32)
            hA = pp.tile([65, 124], f32)
            hB = pp.tile([65, 124], f32)
            ident = pp.tile([128, 128], f32)
            ones = pp.tile([1, 512], f32)
            lin1out = pp.tile([64, 496], f32)

            cst = {}
            for name in ['convA1', 'convA2', 'convB1', 'convB2',
                         'lin1w', 'lin1w2', 'lin1b', 'giW_r', 'giW_z', 'giW_n',
                         'giB_r', 'giB_z', 'giB_n', 'gruW_rz',
                         'gruW_n', 'attw', 'lin3T', 'lin3b', 'clsT', 'clsb',
                         'kill']:
                t = pp.tile(list(din[name].shape), din[name].dtype, name=f'c_{name}')
                nc.sync.dma_start(t, din[name][:, :])
                cst[name] = t

            make_identity(nc, ident)
            nc.vector.memset(ones, 1.0)
            nc.vector.memset(ETm[64:65, :], 1.0)
            nc.vector.memset(hA, 0.0)
            nc.vector.memset(hA[64:65, :], 1.0)
            nc.vector.memset(hB[64:65, :], 1.0)

            # ================= PHASE A =================
            with tc.tile_pool(name='pA', bufs=3) as pa, \
                 tc.tile_pool(name='pAs', bufs=2) as psl, \
                 tc.tile_pool(name='pAp', bufs=3, space='PSUM') as pap, \
                 tc.tile_pool(name='pAl', bufs=1, space='PSUM') as pal:

                def conv_unit(u, margin):
                    dst = Pm if margin else P2g
                    for half in range(2):
                        nk = KHALF if half == 0 else KC - KHALF
                        s1 = psl.tile([128, KHALF * NH], bft, tag='ms1')
                        s2 = psl.tile([81, KHALF * NH], bft, tag='ms2',
                                      padded_shape=[128, KHALF * NH])
                        nc.sync.dma_start(s1, din['slab1'][2 * u + half])
                        nc.sync.dma_start(s2, din['slab2'][2 * u + half])
                        for kp in range(0, nk, 2):   # pairs of k iterations
                            npair = min(2, nk - kp)
                            pwc = pa.tile([128, 2 * NH], bft, tag='pwc')
                            for par in range(npair):
                                kl = kp + par
                                r1 = s1[:, kl * NH:(kl + 1) * NH]
                                r2 = s2[0:81, kl * NH:(kl + 1) * NH]
                                psA = pap.tile([128, NH], f32, tag='cps')
                                psB = pap.tile([128, NH], f32, tag='cps')
                                nc.tensor.matmul(psA, cst['convA1'], r1, start=True, stop=False)
                                nc.tensor.matmul(psA, cst['convA2'], r2, start=False, stop=True)
                                nc.tensor.matmul(psB, cst['convB1'], r1, start=True, stop=False)
                                nc.tensor.matmul(psB, cst['convB2'], r2, start=False, stop=True)
                                pwb = pa.tile([128, NH], bft, tag='pwb')
                                nc.scalar.activation(pwb, psB, AF.Copy)
                                nc.vector.tensor_tensor(
                                    pwc[:, par * NH:(par + 1) * NH], psA, pwb, op=ALU.max)
                            w = npair * NH
                            m2 = pa.tile([128, 2 * NH], bft, tag='m2', name='m2')[:, 0:w - 1]
                            nc.vector.tensor_tensor(m2, pwc[:, 0:w - 1], pwc[:, 1:w], op=ALU.max)
                            m4 = pa.tile([128, 2 * NH], bft, tag='m4', name='m4')[:, 0:w - 3]
                            nc.vector.tensor_tensor(m4, m2[:, 0:w - 3], m2[:, 2:w - 1], op=ALU.max)
                            pooled = pa.tile([128, 2 * NH], bft, tag='pool', name='pooled')[:, 0:w - 4]
                            nc.vector.tensor_tensor(pooled, m4[:, 0:w - 4], pwc[:, 4:w], op=ALU.max)
                            for par in range(npair):
                                k = (kp + par) if half == 0 else KHALF + kp + par
                                src = pooled[:, par * NH:par * NH + 496]
                                if margin:
                                    # only channels 59..63: Pm[w, (c-59)*496+s]
                                    for jj in range(2):
                                        wrow = 2 * k + jj
                                        nc.gpsimd.dma_start(
                                            _ap_mod(Pm[wrow:wrow + 1, 0:1],
                                                    [[496, 5], [1, 496]]),
                                            src[jj * 64 + 59:jj * 64 + 64, :])
                                else:
                                    # top half: P2g[2k+jj, 1 + 496c + s]
                                    nc.gpsimd.dma_start(
                                        _ap_mod(P2g[2 * k:2 * k + 2, 0:1],
                                                [[496, 64], [1, 496]], 1),
                                        src[0:128, :])
                    if not margin and half == 1:
                        # build the +1-shifted bottom half with compute engines
                        # (DMA store duplication saturates the fabric)
                        nc.vector.tensor_scalar_add(
                            P2g[64:122, 0:18000], P2g[0:58, 1:18001], 0.0)
                        nc.scalar.activation(
                            P2g[64:122, 18000:31743], P2g[0:58, 18001:31744],
                            AF.Copy)

                def lin_gi(br, b_loc, margin):
                    n = 32 if margin else 496
                    pl = pal.tile([64, 512], f32, tag='linps', name='pl')[:, 0:n]
                    if margin:
                        for cp in range(64):
                            rhs = _ap_mod(Pm[0:58, 0:1], [[64, n]], 432 + cp)
                            nc.tensor.matmul(pl, cst['lin1w'][:, cp * 64:(cp + 1) * 64],
                                             rhs, start=(cp == 0), stop=False)
                    else:
                        for q in range(32):
                            rhs = _ap_mod(P2g[0:122, 0:1], [[64, n]], 1 + 2 * q)
                            nc.tensor.matmul(pl, cst['lin1w2'][:, q * 64:(q + 1) * 64],
                                             rhs, start=(q == 0), stop=False)
                    nc.tensor.matmul(pl, cst['lin1b'], ones[0:1, 0:n],
                                     start=False, stop=True)
                    lo = lin1out[:, 0:n]
                    nc.scalar.activation(lo, pl, AF.Copy)
                    gparts = [('giW_r', 'giB_r', gi_rz, 0, 'gir'),
                              ('giW_z', 'giB_z', gi_rz, 64, 'giz'),
                              ('giW_n', 'giB_n', gi_n, 0, 'gin')]
                    for wname, bname, store, prow, tg in gparts:
                        pg = pal.tile([64, 512], f32, tag=tg, name='pg_' + tg)[:, 0:n]
                        nc.tensor.matmul(pg, cst[wname], lo, start=True, stop=False)
                        nc.tensor.matmul(pg, cst[bname], ones[0:1, 0:n], start=False, stop=True)
                        if margin:
                            # psum col i = (grp, t): dst col = t*64 + grp
                            nc.scalar.activation(
                                _ap_mod(store[prow:prow + 64, 0:1], [[1, 2], [64, 16]], br * GL),
                                pg, AF.Copy)
                        else:
                            # psum col s' = (j, t): dst col = t*64 + (31*b_loc + j + 2)
                            off = br * GL + 31 * b_loc + 2
                            nc.scalar.activation(
                                _ap_mod(store[prow:prow + 64, 0:1], [[1, 31], [64, 16]], off),
                                _ap_mod(pg, [[16, 31], [1, 16]]), AF.Copy)

                for u, (br, b_loc, margin) in enumerate([
                        (0, 0, True), (0, 0, False), (0, 1, False),
                        (1, 0, True), (1, 0, False), (1, 1, False)]):
                    conv_unit(u, margin)
                    lin_gi(br, b_loc, margin)

            # ================= PHASE B: GRU =================
            # the two branches (eval / template) are independent chains; they
            # are emitted as separate 62-col streams so their steps interleave
            # across engines and hide the per-step handoff latency.
            with tc.tile_pool(name='pB', bufs=2) as pb, \
                 tc.tile_pool(name='pBp', bufs=2, space='PSUM') as pbp:

                def h_ap(br, t, p=65):
                    return _ap_mod(ETm[0:p, 0:1], [[16, 62]], br * 1024 + t)

                def gi_ap(store, br, p, t):
                    if t >= 0:
                        tp, dlt = t, 0
                    elif t >= -16:
                        tp, dlt = 16 + t, -1
                    else:
                        tp, dlt = 32 + t, -2
                    return _ap_mod(store[0:p, 0:1], [[1, 62]],
                                   br * GL + tp * 64 + 2 + dlt)

                killed = pp.tile([65, 124], f32)
                nc.vector.memset(killed[64:65, :], 1.0)

                for i, t in enumerate(range(-W, L)):
                    st = {}
                    for br in range(2):
                        cs = slice(br * 62, br * 62 + 62)
                        if t < 0:
                            h_in = hA if i % 2 == 0 else hB
                            h_out_ap = (hB if i % 2 == 0 else hA)[0:64, cs]
                        elif t == 0:
                            h_in = killed
                            h_out_ap = h_ap(br, 0, 64)
                        else:
                            h_in = None
                            h_out_ap = h_ap(br, t, 64)

                        h_in_ap = h_in[0:65, cs] if h_in is not None else h_ap(br, t - 1)
                        h_in64 = h_in[0:64, cs] if h_in is not None else h_ap(br, t - 1, 64)

                        prz = pbp.tile([128, 62], f32, tag=f'grz{br}', name='prz',
                                       padded_shape=[128, 64])
                        pn = pbp.tile([64, 62], f32, tag=f'gn{br}', name='pn',
                                      padded_shape=[128, 64])
                        r = pb.tile([64, 62], f32, tag=f'r{br}', name='r',
                                    padded_shape=[128, 62])
                        z = pb.tile([64, 62], f32, tag=f'z{br}', name='z',
                                    padded_shape=[128, 62])
                        t2 = pb.tile([64, 62], f32, tag=f't2{br}', name='t2',
                                     padded_shape=[128, 62])
                        npre = pb.tile([64, 62], f32, tag=f'npre{br}', name='npre',
                                       padded_shape=[128, 62])
                        nt = pb.tile([64, 62], f32, tag=f'nt{br}', name='nt',
                                     padded_shape=[128, 62])
                        dmn = pb.tile([64, 62], f32, tag=f'dmn{br}', name='dmn',
                                      padded_shape=[128, 62])
                        e = pb.tile([64, 62], f32, tag=f'e{br}', name='e',
                                    padded_shape=[128, 62])
                        st[br] = (cs, h_in, h_out_ap, h_in_ap, h_in64,
                                  prz, pn, r, z, t2, npre, nt, dmn, e)
                    # stage-ordered emission: same stage for both branches is
                    # adjacent in each engine queue, so the independent chains
                    # pipeline instead of head-of-line blocking.
                    for br in range(2):
                        (cs, h_in, h_out_ap, h_in_ap, h_in64,
                         prz, pn, r, z, t2, npre, nt, dmn, e) = st[br]
                        nc.tensor.matmul(prz, cst['gruW_rz'], h_in_ap, start=True, stop=False)
                        nc.tensor.matmul(prz, ident, gi_ap(gi_rz, br, 128, t),
                                         start=False, stop=True)
                        nc.tensor.matmul(pn, cst['gruW_n'], h_in_ap, start=True, stop=True)
                    for br in range(2):
                        (cs, h_in, h_out_ap, h_in_ap, h_in64,
                         prz, pn, r, z, t2, npre, nt, dmn, e) = st[br]
                        nc.scalar.activation(r, prz[0:64, :], AF.Sigmoid)
                        nc.scalar.activation(z, prz[64:128, :], AF.Sigmoid)
                    for br in range(2):
                        (cs, h_in, h_out_ap, h_in_ap, h_in64,
                         prz, pn, r, z, t2, npre, nt, dmn, e) = st[br]
                        nc.vector.tensor_mul(t2, r, pn)
                    for br in range(2):
                        (cs, h_in, h_out_ap, h_in_ap, h_in64,
                         prz, pn, r, z, t2, npre, nt, dmn, e) = st[br]
                        nc.vector.tensor_add(npre, t2, gi_ap(gi_n, br, 64, t))
                    for br in range(2):
                        (cs, h_in, h_out_ap, h_in_ap, h_in64,
                         prz, pn, r, z, t2, npre, nt, dmn, e) = st[br]
                        nc.scalar.activation(nt, npre, AF.Tanh)
                    for br in range(2):
                        (cs, h_in, h_out_ap, h_in_ap, h_in64,
                         prz, pn, r, z, t2, npre, nt, dmn, e) = st[br]
                        nc.vector.scalar_tensor_tensor(dmn, nt, -1.0, h_in64,
                                                       op0=ALU.mult, op1=ALU.add)
                    for br in range(2):
                        (cs, h_in, h_out_ap, h_in_ap, h_in64,
                         prz, pn, r, z, t2, npre, nt, dmn, e) = st[br]
                        nc.vector.tensor_mul(e, z, dmn)
                    for br in range(2):
                        (cs, h_in, h_out_ap, h_in_ap, h_in64,
                         prz, pn, r, z, t2, npre, nt, dmn, e) = st[br]
                        nc.vector.tensor_add(h_out_ap, nt, e)
                        if t == -1:
                            last = hB if i % 2 == 0 else hA
                            nc.vector.tensor_mul(killed[0:64, cs], last[0:64, cs],
                                                 cst['kill'][:, cs])

            # ================= PHASE C =================
            with tc.tile_pool(name='pC', bufs=2) as pc_, \
                 tc.tile_pool(name='pCe', bufs=4) as pce, \
                 tc.tile_pool(name='pCp', bufs=2, space='PSUM') as pcp, \
                 tc.tile_pool(name='pCs', bufs=1, space='PSUM') as pcs:
                NB = [(0, 128), (128, 128), (256, 128), (384, 112)]
                for b_loc in range(2):
                    Es = ETm[0:64, b_loc * 496:b_loc * 496 + 496]
                    Ts = ETm[0:64, 1024 + b_loc * 496:1024 + b_loc * 496 + 496]
                    etiles, tmts = [], []
                    for (nb0, nbs) in NB:
                        psc = pcp.tile([128, 512], f32, tag='sc', name='psc')[0:nbs, 0:496]
                        nc.tensor.matmul(psc, Ts[:, nb0:nb0 + nbs], Es, start=True, stop=True)
                        nmx = pc_.tile([128, 1], f32, tag='nmx', name='nmx')[0:nbs, :]
                        nc.vector.tensor_reduce(nmx, psc, axis=mybir.AxisListType.X,
                                                op=ALU.max, negate=True)
                        et = pce.tile([128, 496], bft, tag='et', name='et')[0:nbs, :]
                        ssum = pc_.tile([128, 1], f32, tag='ssum', name='ssum')[0:nbs, :]
                        nc.scalar.activation(et, psc, AF.Exp, bias=nmx, accum_out=ssum)
                        rs = pc_.tile([128, 1], f32, tag='rs', name='rs')[0:nbs, :]
                        nc.vector.reciprocal(rs, ssum)
                        ptt = pcp.tile([128, 512], f32, tag='ptt', bufs=1, name='ptt')[0:nbs, 0:64]
                        nc.tensor.transpose(ptt, Ts[:, nb0:nb0 + nbs], ident[0:64, 0:64])
                        tmt = pce.tile([128, 64], bft, tag='tmt', name='tmt')[0:nbs, :]
                        nc.scalar.activation(tmt, ptt, AF.Copy, scale=rs)
                        etiles.append(et)
                        tmts.append(tmt)
                    ptp = pcs.tile([64, 512], f32, tag='tp', padded_shape=[128, 512], name='ptp')[:, 0:496]
                    for q, (nb0, nbs) in enumerate(NB):
                        nc.tensor.matmul(ptp, tmts[q], etiles[q],
                                         start=(q == 0), stop=(q == 3))
                    da = pc_.tile([64, 496], f32, tag='da', padded_shape=[128, 496])
                    nc.vector.tensor_sub(da, ptp, Es)
                    da2 = pc_.tile([64, 496], f32, tag='da2', padded_shape=[128, 496])
                    nc.scalar.activation(da2, da, AF.Abs)
                    patt = pcs.tile([1, 512], f32, tag='chain', padded_shape=[128, 512], bufs=2, name='patt')[:, 0:496]
                    nc.tensor.matmul(patt, cst['attw'], Es, start=True, stop=True)
                    anm = pc_.tile([1, 1], f32, tag='anm', padded_shape=[128, 1])
                    nc.vector.tensor_reduce(anm, patt, axis=mybir.AxisListType.X,
                                            op=ALU.max, negate=True)
                    ea = pc_.tile([1, 496], f32, tag='ea', padded_shape=[128, 496])
                    asum = pc_.tile([1, 1], f32, tag='asum', padded_shape=[128, 1])
                    nc.scalar.activation(ea, patt, AF.Exp, bias=anm, accum_out=asum)
                    ars = pc_.tile([1, 1], f32, tag='ars', padded_shape=[128, 1])
                    nc.vector.reciprocal(ars, asum)
                    pab = pcs.tile([64, 512], f32, tag='pab', padded_shape=[128, 512], name='pab')[:, 0:496]
                    nc.tensor.matmul(pab, ones[0:1, 0:64], ea, start=True, stop=True)
                    junk = pc_.tile([64, 496], f32, tag='junk', padded_shape=[128, 496])
                    nc.vector.tensor_mul(junk, da2, pab)
                    rep = pc_.tile([64, 1], f32, tag='rep', padded_shape=[128, 1])
                    nc.vector.tensor_reduce(rep, junk, axis=mybir.AxisListType.X,
                                            op=ALU.add)
                    prsb = pcs.tile([64, 1], f32, tag='chain', padded_shape=[128, 512], bufs=2)
                    nc.tensor.matmul(prsb, ones[0:1, 0:64], ars, start=True, stop=True)
                    rsb = pc_.tile([64, 1], f32, tag='rsb', padded_shape=[128, 1])
                    nc.scalar.activation(rsb, prsb, AF.Copy)
                    h1 = pc_.tile([64, 1], f32, tag='h1', padded_shape=[128, 1])
                    nc.scalar.activation(h1, rep, AF.Relu, scale=rsb)
                    ph2 = pcs.tile([128, 1], f32, tag='chain', padded_shape=[128, 512], bufs=2)
                    nc.tensor.matmul(ph2, cst['lin3T'], h1, start=True, stop=False)
                    nc.tensor.matmul(ph2, cst['lin3b'], ones[0:1, 0:1], start=False, stop=True)
                    h2 = pc_.tile([128, 1], f32, tag='h2')
                    nc.scalar.activation(h2, ph2, AF.Relu)
                    po = pcs.tile([2, 1], f32, tag='chain', padded_shape=[128, 512], bufs=2)
                    nc.tensor.matmul(po, cst['clsT'], h2, start=True, stop=False)
                    nc.tensor.matmul(po, cst['clsb'], ones[0:1, 0:1], start=False, stop=True)
                    osb = pc_.tile([2, 1], f32, tag='osb', padded_shape=[128, 1])
                    nc.scalar.activation(osb, po, AF.Copy)
                    nc.sync.dma_start(dout[b_loc:b_loc + 1, 0:2], osb)

    return nc


def kernel(**inputs):
    _install_bir_fix()
    inputs = {k: np.asarray(v) for k, v in inputs.items()}
    in_maps = _host_prep(inputs)
    if 'nc' not in _CACHE:
        _CACHE['nc'] = _build_program()
    nc = _CACHE['nc']
    from concourse.bass_utils import run_bass_kernel_spmd
    res = run_bass_kernel_spmd(nc, in_maps, core_ids=list(range(8)))
    out = np.zeros((16, 2), np.float32)
    for c in range(8):
        out[2 * c:2 * c + 2] = res.results[c]['outloc']
    return out
